# revision 53
# baseline (speedup 1.0000x reference)
"""3-layer GAT on 8 TRN2 NeuronCores.

Strategy (graph/data parallel, per sharding hint):
- Nodes sharded 8 ways by id (2500/core, padded to 2560). Each core owns the
  aggregation for its dst nodes; edges partitioned by dst core and sorted by
  (dst_block, src) with per-block padding to multiples of 128. Src-sorted
  order keeps the gather's DRAM fetch stream ascending (~7.5ns/row vs ~10
  random; the fetch fabric is transaction-bound, so this order is the single
  biggest lever).
- Per layer: local feature matmul (xT as stationary operand) produces the
  node's h row plus attention logits al_s/al_d; rows are AllGathered into a
  global [20480, row] table (in 3 chunks with chunk-major row ids, so early
  chunks hide under the previous edge phase and the small last chunk
  minimizes the exposed inter-layer tail); per-edge src rows are fetched
  with dma_gather (SWDGE); per-edge al_d comes from a gather of a compact
  local [NPAD, 128] ald table (local + sorted + repeated ids hit DRAM pages
  at ~1.7ns/row); segment-softmax + weighted aggregation run as one-hot
  matmuls on TensorE with PSUM accumulation per 128-dst block. Bias + relu
  fold into the post-transpose activation on the Scalar engine (bias is
  per-partition in feature-transposed space); the classifier pipelines
  per-block inside layer 3.
- Attention: alpha = exp(lrelu(al_s[src]+al_d[dst])) / segsum(...); the max
  subtraction in the reference softmax is shift-invariant and dropped (|e| is
  bounded ~9 here, exp stays in fp32 range). The +1e-16 on the denominator is
  dropped: every dst has a self-loop, so the denominator >= e^-9 >> 1e-16.
"""
import os
import sys
sys.path.insert(0, "/opt/trn_rl_repo")
import numpy as np
import ml_dtypes

import concourse.bass as bass
import concourse.tile as tile
from concourse import bacc, mybir
from concourse.bass_utils import run_bass_kernel_spmd
from concourse.masks import make_identity

BF16 = ml_dtypes.bfloat16
N = 20000
E = 320000
FIN = 1024
H = 4
C = 64
NCLS = 4
NEG = 0.2
NCORE = 8
NLOC = 2500
NPAD = 2560           # per-core node rows, padded to x128
NBLK = NPAD // 128    # dst blocks per core
NG = NCORE * NPAD     # global padded table rows
ROW12 = 384           # bf16 cols: h 0:256, al_s 256:260, al_d 260:264, pad
ROW3 = 128            # bf16 cols: h 0:64, al_s 64, al_d 65, pad

QS = int(os.environ.get("QS", "1"))       # src gather queue split (1/2/4)
SP = bool(int(os.environ.get("SP", "0")))  # single_packet
GBUFS = int(os.environ.get("GBUFS", "3"))  # gather tile double/triple buffering
DSTMM = bool(int(os.environ.get("DSTMM", "0")))  # al_d via matmul, no dst gather
# (measured: dst-sorted edge order costs more src-gather locality than the
# dst gather costs fabric — keep the dst gather)
# AllGather chunk row boundaries: early chunks hide under the previous edge
# phase; the small last chunk minimizes the exposed inter-layer tail (the
# serial collective stream means a late-starting big chunk queues the final
# chunk past the edge-phase end)
AG_SPLITS = (0, 1024, 1792, 2432, NPAD)

_cache = {}
last = {}


def _prep_edges(src, dst):
    """Partition edges by dst core, sort by (dst block, dst, src), pad per
    block.

    Returns (Q, per_core) where per_core[c] = (idx_s, dloc_idx, dst_rel,
    slo, shi): idx_s [NBLK, EPB] int16 global padded src row ids, dloc_idx
    [NBLK, EPB] int16 local dst row ids, dst_rel [NBLK, EPB] f32 (pad -1),
    slo/shi [128, NBLK] f32 per-dst slot ranges within the block."""
    core = dst // NLOC
    dloc = dst - core * NLOC
    blk = dloc // 128
    per_core_lists = []
    maxq = 0
    for c in range(NCORE):
        m = core == c
        s_c, dl_c, b_c = src[m], dloc[m], blk[m]
        order = np.lexsort((s_c, dl_c, b_c) if DSTMM else (s_c, b_c))
        s_c, dl_c, b_c = s_c[order], dl_c[order], b_c[order]
        counts = np.bincount(b_c, minlength=NBLK)
        maxq = max(maxq, int(np.ceil(counts.max() / 128)))
        per_core_lists.append((s_c, dl_c, b_c, counts))
    Q = maxq
    Q = ((Q + QS - 1) // QS) * QS  # divisible by queue split
    EPB = Q * 128
    out = []
    for c in range(NCORE):
        s_c, dl_c, b_c, counts = per_core_lists[c]
        idx_s = np.zeros((NBLK, EPB), np.int64)
        idx_d = np.zeros((NBLK, EPB), np.int64)
        dst_rel = np.full((NBLK, EPB), -1.0, np.float32)
        slo = np.zeros((128, NBLK), np.float32)
        shi = np.zeros((128, NBLK), np.float32)
        # chunk-major global ids so chunked AllGather outputs stay
        # contiguous: gid = chunk_global_base + core*chunk_rows + row-in-chunk
        base_l = np.array(AG_SPLITS[:-1])
        base_g = np.concatenate([[0], np.cumsum(np.diff(AG_SPLITS) * NCORE)[:-1]])
        pos = 0
        for b in range(NBLK):
            n = counts[b]
            sb = s_c[pos:pos + n]
            db = dl_c[pos:pos + n]
            pos += n
            sc, sl = sb // NLOC, sb % NLOC
            ci = np.searchsorted(np.array(AG_SPLITS), sl, side="right") - 1
            rows = np.diff(AG_SPLITS)[ci]
            idx_s[b, :n] = base_g[ci] + sc * rows + (sl - base_l[ci])
            idx_d[b, :n] = db
            rel = (db % 128).astype(np.int64)
            dst_rel[b, :n] = rel.astype(np.float32)
            if DSTMM:
                # dst-sorted: per-dst slot ranges [lo, hi)
                cnt = np.bincount(rel, minlength=128)
                ends = np.cumsum(cnt)
                slo[:, b] = ends - cnt
                shi[:, b] = ends
        out.append((idx_s.astype(np.int16), idx_d.astype(np.int16), dst_rel,
                    slo, shi))
    return Q, out


def _wrap_idx(idx):
    """[NBLK, EPB] -> dma_gather layout [128, NBLK*EPB//16] int16
    (entry i of block b at [i%16, b*EPB//16 + i//16], tiled x8 partitions)."""
    nblk, epb = idx.shape
    w = np.zeros((16, nblk * (epb // 16)), np.int16)
    cols = epb // 16
    for b in range(nblk):
        blkv = idx[b]
        w[:, b * cols:(b + 1) * cols] = blkv.reshape(cols, 16).T
    return np.tile(w, (8, 1))


def _pack_drel(dst_rel, Q):
    """[NBLK, EPB] -> [128, NBLK*Q] with [p, b*Q+s] = edge (b, s*128+p)."""
    nblk, epb = dst_rel.shape
    out = np.zeros((128, nblk * Q), np.float32)
    for b in range(nblk):
        out[:, b * Q:(b + 1) * Q] = dst_rel[b].reshape(Q, 128).T
    return out.astype(BF16)


def _fold_w(W, a_s, a_d):
    """[F, H*C] weights + per-head a vectors -> [F, H*C + 2H] f32."""
    F = W.shape[0]
    Hh, Cc = a_s.shape
    As = np.zeros((Hh * Cc, Hh), np.float64)
    Ad = np.zeros((Hh * Cc, Hh), np.float64)
    for h in range(Hh):
        As[h * Cc:(h + 1) * Cc, h] = a_s[h]
        Ad[h * Cc:(h + 1) * Cc, h] = a_d[h]
    W64 = W.astype(np.float64)
    return np.concatenate([W64, W64 @ As, W64 @ Ad], axis=1).astype(np.float32)


def _build(Q):
    dt = mybir.dt
    nc = bacc.Bacc("TRN2", num_devices=NCORE, debug=False, num_swdge_queues=4)

    xt_in = nc.dram_tensor("xt", [FIN, NPAD], dt.bfloat16, kind="ExternalInput")
    w1e_in = nc.dram_tensor("w1e", [FIN, 264], dt.bfloat16, kind="ExternalInput")
    w2e_in = nc.dram_tensor("w2e", [256, 264], dt.bfloat16, kind="ExternalInput")
    w3e_in = nc.dram_tensor("w3e", [256, 66], dt.bfloat16, kind="ExternalInput")
    wc_in = nc.dram_tensor("wc", [64, 4], dt.bfloat16, kind="ExternalInput")
    b1_in = nc.dram_tensor("b1r", [128, 2], dt.bfloat16, kind="ExternalInput")
    b2_in = nc.dram_tensor("b2r", [128, 2], dt.bfloat16, kind="ExternalInput")
    b3_in = nc.dram_tensor("b3r", [64, 1], dt.bfloat16, kind="ExternalInput")
    bc_in = nc.dram_tensor("bcr", [128, 4], dt.float32, kind="ExternalInput")
    iota_in = nc.dram_tensor("iota", [128, 128], dt.bfloat16, kind="ExternalInput")
    ixs_in = nc.dram_tensor("ixs", [128, NBLK * Q * 8], dt.int16, kind="ExternalInput")
    ixd_in = nc.dram_tensor("ixd", [128, NBLK * Q * 8], dt.int16, kind="ExternalInput")
    drel_in = nc.dram_tensor("drel", [128, NBLK * Q], dt.bfloat16, kind="ExternalInput")
    iotae_in = nc.dram_tensor("iotae", [128, Q * 128], dt.float32, kind="ExternalInput")
    slo_in = nc.dram_tensor("slo", [128, NBLK], dt.float32, kind="ExternalInput")
    shi_in = nc.dram_tensor("shi", [128, NBLK], dt.float32, kind="ExternalInput")
    out_d = nc.dram_tensor("out", [NPAD, 4], dt.float32, kind="ExternalOutput")

    tabin = [nc.dram_tensor(f"tabin{l}", [NPAD, r], dt.bfloat16, kind="Internal")
             for l, r in ((1, ROW12), (2, ROW12), (3, ROW3))]
    tabg = [nc.dram_tensor(f"tabg{l}", [NG, r], dt.bfloat16, kind="Internal",
                           addr_space="Shared")
            for l, r in ((1, ROW12), (2, ROW12), (3, ROW3))]
    # compact per-layer local ald tables (256B rows) for the dst-side gather
    aldt = [nc.dram_tensor(f"aldt{l}", [NPAD, 128], dt.bfloat16, kind="Internal")
            for l in (1, 2, 3)]

    with tile.TileContext(nc) as tc:
        with (
            tc.tile_pool(name="const", bufs=1) as cpool,
            tc.tile_pool(name="work", bufs=2) as wpool,
            tc.tile_pool(name="psum", bufs=2, space="PSUM") as ppool,
        ):
            # ---- constants to SBUF
            ident = cpool.tile([128, 128], dt.bfloat16)
            make_identity(nc, ident[:])
            iota = cpool.tile([128, 128], dt.bfloat16, tag="iota")
            nc.sync.dma_start(out=iota[:], in_=iota_in[:])
            ixs = cpool.tile([128, NBLK * Q * 8], dt.int16, tag="ixs")
            nc.sync.dma_start(out=ixs[:], in_=ixs_in[:])
            if not DSTMM:
                ixd = cpool.tile([128, NBLK * Q * 8], dt.int16, tag="ixd")
                nc.sync.dma_start(out=ixd[:], in_=ixd_in[:])
            drel = cpool.tile([128, NBLK * Q], dt.bfloat16, tag="drel")
            nc.sync.dma_start(out=drel[:], in_=drel_in[:])
            if DSTMM:
                iotae = cpool.tile([128, Q * 128], dt.float32, tag="iotae")
                nc.sync.dma_start(out=iotae[:], in_=iotae_in[:])
                slo = cpool.tile([128, NBLK], dt.float32, tag="slo")
                nc.sync.dma_start(out=slo[:], in_=slo_in[:])
                shi = cpool.tile([128, NBLK], dt.float32, tag="shi")
                nc.sync.dma_start(out=shi[:], in_=shi_in[:])
            # per-layer local al_d, stashed from phase_a (partition = dst%128)
            ald_all = [cpool.tile([128, NBLK * H], dt.bfloat16, tag=f"ald{l}",
                                  name=f"ald_all{l}")
                       for l in range(3)]
            w1e = cpool.tile([128, 8, 264], dt.bfloat16, tag="w1e")
            nc.sync.dma_start(out=w1e[:], in_=w1e_in[:].rearrange("(k p) c -> p k c", p=128))
            w2e = cpool.tile([128, 2, 264], dt.bfloat16, tag="w2e")
            nc.sync.dma_start(out=w2e[:], in_=w2e_in[:].rearrange("(k p) c -> p k c", p=128))
            w3e = cpool.tile([128, 2, 66], dt.bfloat16, tag="w3e")
            nc.sync.dma_start(out=w3e[:], in_=w3e_in[:].rearrange("(k p) c -> p k c", p=128))
            wc = cpool.tile([64, 4], dt.bfloat16, tag="wc")
            nc.sync.dma_start(out=wc[:], in_=wc_in[:])
            # biases in transposed space: per-feature = per-partition scalar,
            # folded into the post-transpose activation on ACT
            b1r = cpool.tile([128, 2], dt.bfloat16, tag="b1r")
            nc.sync.dma_start(out=b1r[:], in_=b1_in[:])
            b2r = cpool.tile([128, 2], dt.bfloat16, tag="b2r")
            nc.sync.dma_start(out=b2r[:], in_=b2_in[:])
            b3r = cpool.tile([64, 1], dt.bfloat16, tag="b3r")
            nc.sync.dma_start(out=b3r[:], in_=b3_in[:])
            bcr = cpool.tile([128, 4], dt.float32, tag="bcr")
            nc.sync.dma_start(out=bcr[:], in_=bc_in[:])

            # xT for layer 1 streamed from DRAM into SBUF once (in halves so
            # phase_a's first blocks start before the whole load finishes)
            xt1 = cpool.tile([128, 8, NPAD], dt.bfloat16, tag="xt1")
            xtv = xt_in[:].rearrange("(k p) n -> p k n", p=128)
            nc.sync.dma_start(out=xt1[:, :, 0:NPAD // 2], in_=xtv[:, :, 0:NPAD // 2])
            nc.sync.dma_start(out=xt1[:, :, NPAD // 2:], in_=xtv[:, :, NPAD // 2:])
            # xT buffers for layers 2/3 outputs
            xt2 = cpool.tile([128, 2, NPAD], dt.bfloat16, tag="xt2")
            xt2b = cpool.tile([128, 2, NPAD], dt.bfloat16, tag="xt2b")
            xt3 = cpool.tile([64, NPAD], dt.bfloat16, tag="xt3")

            def phase_a_block(m, xt_sb, kc, wext, cols, tab_in, ald_t, ald_sb,
                              row, hcols, acols):
                """h|al = x @ Wext for one 128-node chunk; write table + ald."""
                hh = acols // 2
                ps = ppool.tile([128, cols], dt.float32, tag="psA",
                                bufs=1 if DSTMM else 2)
                for k in range(kc):
                    lhsT = (xt_sb[:, k, m * 128:(m + 1) * 128] if kc > 1
                            else xt_sb[:, m * 128:(m + 1) * 128])
                    nc.tensor.matmul(ps[:], lhsT, wext[:, k, :] if kc > 1 else wext[:],
                                     start=(k == 0), stop=(k == kc - 1))
                hrow = wpool.tile([128, row], dt.bfloat16, tag="hrow")
                nc.scalar.copy(hrow[:, 0:hcols + acols], ps[:, 0:hcols + acols])
                nc.sync.dma_start(out=tab_in[m * 128:(m + 1) * 128, 0:hcols + acols],
                                  in_=hrow[:, 0:hcols + acols])
                if DSTMM:
                    nc.vector.tensor_copy(ald_sb[:, m * hh:(m + 1) * hh],
                                          hrow[:, hcols + hh:hcols + acols])
                else:
                    nc.sync.dma_start(out=ald_t[m * 128:(m + 1) * 128, 0:hh],
                                      in_=hrow[:, hcols + hh:hcols + acols])

            def allgather_chunk(j, tin, tg):
                r0, r1 = AG_SPLITS[j], AG_SPLITS[j + 1]
                gbase = NCORE * r0
                nc.gpsimd.collective_compute(
                    "AllGather", mybir.AluOpType.bypass,
                    replica_groups=[list(range(NCORE))],
                    ins=[tin[r0:r1, :]],
                    outs=[tg[gbase:gbase + NCORE * (r1 - r0), :]])

            def edge_block(b, l, tab, ald_t, ald_sb, row, hcols, hh, xt_out,
                           brep, do_relu, classify=False):
                """One dst-block: gather rows, attention, one-hot matmul agg."""
                mcols = hcols + hh      # matmul rhs/psum cols (msg | w)
                if True:
                    g = wpool.tile([128, Q, row], dt.bfloat16, tag="g", bufs=GBUFS)
                    per = Q // QS * 128
                    for j in range(QS):
                        nc.gpsimd.dma_gather(
                            out_ap=g[:, j * (Q // QS):(j + 1) * (Q // QS), :],
                            in_ap=tab[:],
                            idxs_ap=ixs[:, b * Q * 8 + j * (per // 16):
                                        b * Q * 8 + (j + 1) * (per // 16)],
                            num_idxs=per, num_idxs_reg=per, elem_size=row,
                            single_packet=SP,
                            queue_num=(b % (4 if DSTMM else 3)) if QS == 1 else j)
                    if DSTMM:
                        # sblkT[d, (s,p)] = lo_d <= slot < hi_d (dst-sorted
                        # edges); eald[e, h] = sum_d sblkT[d, e] * ald[d, h]
                        sbt = wpool.tile([128, Q, 128], dt.bfloat16, tag="sbt")
                        nc.vector.tensor_scalar(
                            out=sbt[:].rearrange("p q e -> p (q e)"),
                            in0=iotae[:], scalar1=slo[:, b:b + 1],
                            scalar2=None, op0=mybir.AluOpType.is_ge)
                        nc.vector.scalar_tensor_tensor(
                            out=sbt[:].rearrange("p q e -> p (q e)"),
                            in0=iotae[:], scalar=shi[:, b:b + 1],
                            in1=sbt[:].rearrange("p q e -> p (q e)"),
                            op0=mybir.AluOpType.is_lt,
                            op1=mybir.AluOpType.mult)
                        pse = ppool.tile([128, Q * hh], dt.float32, tag="psE",
                                         bufs=1)
                        for s in range(Q):
                            nc.tensor.matmul(
                                pse[:, s * hh:(s + 1) * hh], sbt[:, s, :],
                                ald_sb[:, b * hh:(b + 1) * hh],
                                start=True, stop=True)
                        ald_edge = pse[:].rearrange("p (q h) -> p q h", h=hh)
                    else:
                        gd = wpool.tile([128, Q, 128], dt.bfloat16, tag="gd",
                                        bufs=GBUFS)
                        nc.gpsimd.dma_gather(
                            out_ap=gd[:], in_ap=ald_t[:],
                            idxs_ap=ixd[:, b * Q * 8:(b + 1) * Q * 8],
                            num_idxs=Q * 128, num_idxs_reg=Q * 128, elem_size=128,
                            single_packet=SP,
                            queue_num=3 if QS == 1 else (QS + b) % 4)
                        ald_edge = gd[:, :, 0:hh]
                    # e = lrelu(al_s[src] + al_d[dst]); w = exp(e) into g's al cols
                    ew = wpool.tile([128, Q * hh], dt.float32, tag="ew")
                    nc.vector.tensor_tensor(
                        out=ew[:].rearrange("p (q h) -> p q h", h=hh),
                        in0=g[:, :, hcols:hcols + hh],
                        in1=ald_edge,
                        op=mybir.AluOpType.add)
                    nc.vector.scalar_tensor_tensor(
                        out=ew[:], in0=ew[:], scalar=NEG, in1=ew[:],
                        op0=mybir.AluOpType.mult, op1=mybir.AluOpType.max)
                    nc.scalar.activation(g[:, :, hcols:hcols + hh],
                                         ew[:].rearrange("p (q h) -> p q h", h=hh),
                                         mybir.ActivationFunctionType.Exp)
                    # S block: one-hot [e, dst_rel]
                    sblk = wpool.tile([128, Q, 128], dt.bfloat16, tag="sblk")
                    nc.vector.tensor_tensor(
                        out=sblk[:],
                        in0=iota[:][:, None, :].to_broadcast([128, Q, 128]),
                        in1=drel[:, b * Q:(b + 1) * Q][:, :, None].to_broadcast([128, Q, 128]),
                        op=mybir.AluOpType.is_equal)
                    # scale h by w (bcast per head)
                    nc.vector.tensor_tensor(
                        out=g[:, :, 0:hcols].rearrange("p q (h c) -> p q h c", c=C),
                        in0=g[:, :, 0:hcols].rearrange("p q (h c) -> p q h c", c=C),
                        in1=g[:, :, hcols:hcols + hh][:, :, :, None]
                        .to_broadcast([128, Q, hh, C]),
                        op=mybir.AluOpType.mult)
                    ps = ppool.tile([128, mcols], dt.float32, tag="psC", bufs=3)
                    for s in range(Q):
                        nc.tensor.matmul(ps[:], sblk[:, s, :], g[:, s, 0:mcols],
                                         start=(s == 0), stop=(s == Q - 1))
                    # normalize (denominator >= self-loop weight >> 0)
                    den = wpool.tile([128, hh], dt.float32, tag="den")
                    nc.vector.reciprocal(den[:], ps[:, hcols:mcols])
                    x2 = wpool.tile([128, hcols], dt.bfloat16, tag="x2")
                    nc.vector.tensor_tensor(
                        out=x2[:].rearrange("p (h c) -> p h c", c=C),
                        in0=ps[:, 0:hcols].rearrange("p (h c) -> p h c", c=C),
                        in1=den[:][:, :, None].to_broadcast([128, hh, C]),
                        op=mybir.AluOpType.mult)
                    # transpose into xt_out columns; bias + relu fold into the
                    # post-transpose activation (bias is per-partition there)
                    act_fn = (mybir.ActivationFunctionType.Relu if do_relu
                              else mybir.ActivationFunctionType.Identity)
                    nhalf = hcols // 128
                    if nhalf == 0:
                        tp = ppool.tile([hcols, 128], dt.bfloat16, tag="tp")
                        nc.tensor.transpose(tp[:], x2[:], ident[:])
                        nc.scalar.activation(xt_out[:, b * 128:(b + 1) * 128],
                                             tp[:], act_fn, bias=brep[:, 0:1])
                        if classify:
                            psd = ppool.tile([128, 4], dt.float32, tag="psD",
                                             bufs=1)
                            nc.tensor.matmul(psd[:],
                                             xt_out[:, b * 128:(b + 1) * 128],
                                             wc[:], start=True, stop=True)
                            of = wpool.tile([128, 4], dt.float32, tag="of")
                            nc.vector.tensor_tensor(out=of[:], in0=psd[:],
                                                    in1=bcr[:],
                                                    op=mybir.AluOpType.add)
                            nc.sync.dma_start(
                                out=out_d[b * 128:(b + 1) * 128, :], in_=of[:])
                    else:
                        for hf in range(nhalf):
                            tp = ppool.tile([128, 128], dt.bfloat16, tag="tp")
                            nc.tensor.transpose(tp[:], x2[:, hf * 128:(hf + 1) * 128],
                                                ident[:])
                            nc.scalar.activation(
                                xt_out[:, hf, b * 128:(b + 1) * 128], tp[:],
                                act_fn, bias=brep[:, hf:hf + 1])

            # chunk-end blocks for the AllGather triggers
            chb = [r // 128 for r in AG_SPLITS[1:]]

            # ===== layer 1 phase_a + its AllGather
            for m in range(NBLK):
                phase_a_block(m, xt1, 8, w1e, 264, tabin[0], aldt[0],
                              ald_all[0], ROW12, 256, 8)
                for j, eb in enumerate(chb):
                    if m + 1 == eb:
                        allgather_chunk(j, tabin[0], tabg[0])

            # ===== layer 1 edge phase, layer 2 phase_a/AG interleaved per
            # block so phase_a matmuls land early in the PE queue and each
            # AllGather chunk fires as soon as its input rows exist
            for b in range(NBLK):
                edge_block(b, 1, tabg[0], aldt[0], ald_all[0], ROW12, 256, H,
                           xt2, b1r, True)
                phase_a_block(b, xt2, 2, w2e, 264, tabin[1], aldt[1],
                              ald_all[1], ROW12, 256, 8)
                for j, eb in enumerate(chb):
                    if b + 1 == eb:
                        allgather_chunk(j, tabin[1], tabg[1])

            # ===== layer 2 edge phase, layer 3 phase_a/AG interleaved
            for b in range(NBLK):
                edge_block(b, 2, tabg[1], aldt[1], ald_all[1], ROW12, 256, H,
                           xt2b, b2r, True)
                phase_a_block(b, xt2b, 2, w3e, 66, tabin[2], aldt[2],
                              ald_all[2], ROW3, 64, 2)
                for j, eb in enumerate(chb):
                    if b + 1 == eb:
                        allgather_chunk(j, tabin[2], tabg[2])

            # ===== layer 3 edge phase (1 head, no concat, no relu) +
            # per-block classifier
            for b in range(NBLK):
                edge_block(b, 3, tabg[2], aldt[2], ald_all[2], ROW3, 64, 1,
                           xt3, b3r, False, classify=True)
    nc.compile()
    return nc


def kernel(x, edge_index, w1, as1, ad1, b1, w2, as2, ad2, b2,
           w3, as3, ad3, b3, wc, bc):
    x = np.asarray(x)
    ei = np.asarray(edge_index).astype(np.int64)
    loop = np.arange(N, dtype=np.int64)
    src = np.concatenate([ei[0], loop])
    dst = np.concatenate([ei[1], loop])

    Q, edge_data = _prep_edges(src, dst)

    w1e = _fold_w(np.asarray(w1), np.asarray(as1), np.asarray(ad1)).astype(BF16)
    w2e = _fold_w(np.asarray(w2), np.asarray(as2), np.asarray(ad2)).astype(BF16)
    w3e = _fold_w(np.asarray(w3), np.asarray(as3), np.asarray(ad3)).astype(BF16)
    wc_b = np.asarray(wc).astype(BF16)
    # biases laid out for the transposed (feature-partition) space
    b1r = np.asarray(b1).astype(BF16).reshape(2, 128).T.copy()
    b2r = np.asarray(b2).astype(BF16).reshape(2, 128).T.copy()
    b3r = np.asarray(b3).astype(BF16).reshape(64, 1).copy()
    bcr = np.tile(np.asarray(bc).astype(np.float32)[None, :], (128, 1))
    iota = np.tile(np.arange(128, dtype=np.float32)[None, :], (128, 1)).astype(BF16)

    key = ("k", Q, QS, SP, GBUFS, AG_SPLITS, DSTMM)
    if key not in _cache:
        _cache[key] = _build(Q)
    nc = _cache[key]

    iotae = np.tile(np.arange(Q * 128, dtype=np.float32)[None, :], (128, 1))
    in_maps = []
    for c in range(NCORE):
        idx_s, idx_d, dst_rel, slo, shi = edge_data[c]
        xt = np.zeros((FIN, NPAD), BF16)
        xt[:, :NLOC] = x[c * NLOC:(c + 1) * NLOC].T.astype(BF16)
        in_maps.append({
            "xt": xt, "w1e": w1e, "w2e": w2e, "w3e": w3e, "wc": wc_b,
            "b1r": b1r, "b2r": b2r, "b3r": b3r, "bcr": bcr, "iota": iota,
            "ixs": _wrap_idx(idx_s), "ixd": _wrap_idx(idx_d),
            "drel": _pack_drel(dst_rel, Q),
            "iotae": iotae, "slo": slo, "shi": shi,
        })
    res = run_bass_kernel_spmd(nc, in_maps, core_ids=list(range(NCORE)),
                               trace=bool(os.environ.get("KTRACE")),
                               tmpdir=os.environ.get("KTRACE_DIR") or None)
    last["res"] = res
    out = np.concatenate([res.results[c]["out"][:NLOC] for c in range(NCORE)],
                         axis=0)
    return out.astype(np.float32)
